# revision 9
# baseline (speedup 1.0000x reference)
"""DNDT forward kernel for Trainium2 (8 NeuronCores, data-parallel).

Math (matches the reference):
    w = [1,2,3,4];  b = [0, cumsum(-sort(beta))]
    sigma[i,f,k] = sigmoid((x[i,f]*w[k] + b[k]) / T)            [B, 6, 4]
    leaves[i]    = kron(sigma[i,0], ..., sigma[i,5])            [B, 4096]
    out          = leaves @ L                                   [B, 10]

Log-space restructuring: with sp[i,(f,k)] = softplus(-(x w' + b')) =
-log sigma, every 256-wide Bm row (features 2..5) is
    Bm[i,b] = exp(-sum_f sp[i, (f, k_f(b))])
so the 4-sigmoid product per leaf becomes a 0/-1 pattern MATMUL over the
transposed sp (K=16 active rows) accumulated in PSUM, followed by a
single exp.  The per-row kron chain on the DVE disappears entirely:
    sp   = ln(1 + exp(-z))           scalar engine (one act table)
    spT  = transpose(sp[:, 8:24])    one PE transpose per 1024 rows
    S    = P.T @ spT                 PE, K=128 (16 live), N=128
    BmT  = exp(S)                    scalar, [128,2,128] per row-group
    M    = BmT.T @ L3                PE, K=256, N=160 (fp16)
    out  = sum_a exp(-(sp0+sp1)) * M[(a,c)]   gpsimd/vector epilogue

Per-core layout: 8192 rows as 8 supertiles of 1024; partition p holds
rows {base + p*8 + g}; matmul row-group g covers partitions 0..127.
"""

import numpy as np

import concourse.bacc as bacc
import concourse.mybir as mybir
import concourse.tile as tile
from concourse.bass_utils import run_bass_kernel_spmd

F32 = mybir.dt.float32
F16 = mybir.dt.float16

B, F, NB, NCLS = 65536, 6, 4, 10
CORES = 8
ROWS = B // CORES          # 8192 rows per core
G = 8                      # row-groups (matmul tiles) per supertile
ST_ROWS = 128 * G          # 1024 rows per supertile
N_ST = ROWS // ST_ROWS     # 8 supertiles
TEMP = 0.1

_NC_CACHE = {}


def _build_nc():
    nc = bacc.Bacc("TRN2", target_bir_lowering=False, debug=False)

    xc = nc.dram_tensor("xc", [ROWS, F], F32, kind="ExternalInput")
    wt = nc.dram_tensor("wt", [128, 24], F32, kind="ExternalInput")
    bt = nc.dram_tensor("bt", [128, 24], F32, kind="ExternalInput")
    ident = nc.dram_tensor("ident", [128, 128], F16, kind="ExternalInput")
    pmat = nc.dram_tensor("pmat", [128, G, 2, 128], F16, kind="ExternalInput")
    l3p = nc.dram_tensor("l3p", [128, 2, 160], F16, kind="ExternalInput")
    outc = nc.dram_tensor("outc", [ROWS, NCLS], F32, kind="ExternalOutput")

    EXP = mybir.ActivationFunctionType.Exp
    LN = mybir.ActivationFunctionType.Ln

    with tile.TileContext(nc) as tc:
        with (
            tc.tile_pool(name="consts", bufs=1) as consts,
            tc.tile_pool(name="io", bufs=3) as io,
            tc.tile_pool(name="work", bufs=3) as work,
            tc.tile_pool(name="bmt", bufs=3) as bmtp,
            tc.tile_pool(name="ps_t", bufs=2, space="PSUM") as ps_t,
            tc.tile_pool(name="ps_s", bufs=2, space="PSUM") as ps_s,
            tc.tile_pool(name="ps_m", bufs=2, space="PSUM") as ps_m,
        ):
            wt_sb = consts.tile([128, 24], F32)
            nc.sync.dma_start(wt_sb[:, :], wt[:, :])
            bt_sb = consts.tile([128, 24], F32)
            nc.sync.dma_start(bt_sb[:, :], bt[:, :])
            id_sb = consts.tile([128, 128], F16)
            nc.sync.dma_start(id_sb[:, :], ident[:, :])
            p_sb = consts.tile([128, G, 2, 128], F16)
            nc.sync.dma_start(p_sb[:, :, :, :], pmat[:, :, :, :])
            l3_sb = consts.tile([128, 2, 160], F16)
            nc.sync.dma_start(l3_sb[:, :, :], l3p[:, :, :])

            for st in range(N_ST):
                base = st * ST_ROWS
                xs = xc[base:base + ST_ROWS, :].rearrange("(p g) f -> p g f", g=G)
                x_sb = io.tile([128, G, F], F32, tag="x")
                nc.sync.dma_start(x_sb[:, :, :], xs)

                # z[p,g,f,k] = x[p,g,f]*(w[k]/T) + b[k]/T      [128,8,6,4]
                z = work.tile([128, G, F, NB], F32, tag="z")
                x_b = x_sb[:, :, :].unsqueeze(3).broadcast_to((128, G, F, NB))
                wt_b = (
                    wt_sb[:, :].rearrange("p (f k) -> p f k", k=NB)
                    .unsqueeze(1).broadcast_to((128, G, F, NB))
                )
                bt_b = (
                    bt_sb[:, :].rearrange("p (f k) -> p f k", k=NB)
                    .unsqueeze(1).broadcast_to((128, G, F, NB))
                )
                nc.gpsimd.tensor_mul(z[:, :, :, :], x_b, wt_b)
                nc.gpsimd.tensor_add(z[:, :, :, :], z[:, :, :, :], bt_b)
                # keep z inside the hw exp-table range
                nc.gpsimd.tensor_scalar(
                    z[:, :, :, :], z[:, :, :, :], -87.0, 87.0,
                    op0=mybir.AluOpType.max, op1=mybir.AluOpType.min,
                )

                # sp = ln(1 + min(exp(-z), 3e38))  (= softplus(-z) = -ln sigma)
                t_sb = work.tile([128, G, 24], F32, tag="t")
                nc.scalar.activation(
                    t_sb[:, :, :],
                    z[:, :, :, :].rearrange("p g f k -> p g (f k)"),
                    EXP, scale=-1.0)
                u_sb = work.tile([128, G, 24], F32, tag="u")
                nc.vector.tensor_scalar(
                    u_sb[:, :, :], t_sb[:, :, :], 3.58e9, 1.0,
                    op0=mybir.AluOpType.min, op1=mybir.AluOpType.add,
                )
                # Bm-side (features 2..5) and A-side (features 0,1) softplus
                # in separate tiles so matmul APs flatten to one free dim.
                sp16 = work.tile([128, G, 16], F16, tag="sp16")
                nc.scalar.activation(sp16[:, :, :], u_sb[:, :, 8:24], LN)
                sp = work.tile([128, G, 8], F16, tag="sp8")
                nc.scalar.activation(sp[:, :, :], u_sb[:, :, 0:8], LN)

                # transpose sp16 -> spT[(g,j), p]  [128,128]
                tp = ps_t.tile([128, 128], F16, tag="tp")
                nc.tensor.transpose(
                    tp[:, :],
                    sp16[:, :, :],
                    id_sb[:, :],
                )
                spt = work.tile([128, 128], F16, tag="spt")
                nc.vector.tensor_copy(
                    spt[:, :].bitcast(mybir.dt.uint32),
                    tp[:, :].bitcast(mybir.dt.uint32),
                )

                # A-side: A[p,g,a] = exp(-(sp[f0,k0] + sp[f1,k1]))  [128,8,16]
                sa = work.tile([128, G, 16], F16, tag="sa")
                nc.vector.tensor_add(
                    sa[:, :, :].rearrange("p g (i j) -> p g i j", j=NB),
                    sp[:, :, 0:4].unsqueeze(3).broadcast_to((128, G, NB, NB)),
                    sp[:, :, 4:8].unsqueeze(2).broadcast_to((128, G, NB, NB)),
                )
                ap = work.tile([128, G, 16], F16, tag="ap")
                nc.scalar.activation(ap[:, :, :], sa[:, :, :], EXP, scale=-1.0)

                oq = io.tile([128, G, NCLS], F32, tag="oq")
                prod = work.tile([128, G, 160], F16, tag="prod")
                for half in range(2):
                    # S[b] = -(sum of 4 softplus)  via 0/-1 pattern matmul
                    psS = ps_s.tile([128, G // 2, 2, 128], F32, tag="s")
                    for gg in range(G // 2):
                        g = half * 4 + gg
                        for i in range(2):
                            nc.tensor.matmul(
                                psS[:, gg, i, :], p_sb[:, g, i, :], spt[:, :],
                                start=True, stop=True,
                            )
                    # BmT = exp(S)   [128, 4, 2, 128] fp16
                    bmt = bmtp.tile([128, G // 2, 2, 128], F16, tag="bmt")
                    nc.scalar.activation(
                        bmt[:, :, :, :], psS[:, :, :, :], EXP)
                    for gg in range(G // 2):
                        g = half * 4 + gg
                        mm = ps_m.tile([128, 160], F32, tag="m")
                        nc.tensor.matmul(
                            mm[:, :], bmt[:, gg, 0, :], l3_sb[:, 0, :],
                            start=True, stop=False,
                        )
                        nc.tensor.matmul(
                            mm[:, :], bmt[:, gg, 1, :], l3_sb[:, 1, :],
                            start=False, stop=True,
                        )
                        # prod[p,(a,c)] = A[p,a] * M[p,(a,c)]
                        nc.vector.tensor_mul(
                            prod[:, g, :].rearrange("p (a c) -> p a c", c=NCLS),
                            ap[:, g, :].unsqueeze(2).broadcast_to((128, 16, NCLS)),
                            mm[:, :].rearrange("p (a c) -> p a c", c=NCLS),
                        )

                # reduce over a (16) -> [128, G, 10]
                f1 = work.tile([128, G, 80], F16, tag="f1")
                nc.gpsimd.tensor_add(f1[:, :, :], prod[:, :, 0:80], prod[:, :, 80:160])
                f2 = work.tile([128, G, 40], F16, tag="f2")
                nc.gpsimd.tensor_add(f2[:, :, :], f1[:, :, 0:40], f1[:, :, 40:80])
                nc.vector.tensor_reduce(
                    oq[:, :, :],
                    f2[:, :, :].rearrange("p g (a c) -> p g c a", c=NCLS),
                    axis=mybir.AxisListType.X,
                    op=mybir.AluOpType.add,
                )

                od = outc[base:base + ST_ROWS, :].rearrange("(p g) c -> p g c", g=G)
                nc.sync.dma_start(od, oq[:, :, :])

    nc.compile()
    return nc


def _host_prep(x, beta, leaves2classes):
    x = np.ascontiguousarray(np.asarray(x, dtype=np.float32))
    beta = np.asarray(beta, dtype=np.float32)
    L = np.asarray(leaves2classes, dtype=np.float32)

    w = np.linspace(1.0, float(NB), NB, dtype=np.float32)
    bs = np.sort(beta)
    b = np.concatenate([np.zeros(1, np.float32), np.cumsum(-bs, dtype=np.float32)])

    wt24 = np.tile(w / np.float32(TEMP), F).astype(np.float32)
    bt24 = np.tile(b / np.float32(TEMP), F).astype(np.float32)
    WT = np.ascontiguousarray(np.broadcast_to(wt24, (128, 24)))
    BT = np.ascontiguousarray(np.broadcast_to(bt24, (128, 24)))

    # P[(g,j), g, i, p] = -1 where j = (f-2)*4 + k_f(i*128+p)
    P = np.zeros((128, G, 2, 128), dtype=np.float16)
    bb = np.arange(256)
    digs = np.stack([(bb >> 6) & 3, (bb >> 4) & 3, (bb >> 2) & 3, bb & 3])
    for g in range(G):
        for i in range(2):
            for p in range(128):
                for f in range(4):
                    j = f * 4 + digs[f, i * 128 + p]
                    P[g * 16 + j, g, i, p] = -1.0

    # l3[p, i, a*10+c] = L[a*256 + i*128 + p, c]
    L3 = L.reshape(16, 2, 128, NCLS)           # [a, i, p, c]
    L3P = np.ascontiguousarray(
        L3.transpose(2, 1, 0, 3).reshape(128, 2, 160)).astype(np.float16)

    ident = np.eye(128, dtype=np.float16)
    return x, WT, BT, ident, P, L3P


def kernel(x, beta, leaves2classes):
    x, WT, BT, ident, P, L3P = _host_prep(x, beta, leaves2classes)

    if "nc" not in _NC_CACHE:
        _NC_CACHE["nc"] = _build_nc()
    nc = _NC_CACHE["nc"]

    in_maps = []
    for c in range(CORES):
        in_maps.append({
            "xc": np.ascontiguousarray(x[c * ROWS:(c + 1) * ROWS]),
            "wt": WT,
            "bt": BT,
            "ident": ident,
            "pmat": P,
            "l3p": L3P,
        })
    res = run_bass_kernel_spmd(nc, in_maps, core_ids=list(range(CORES)))
    out = np.concatenate([r["outc"] for r in res.results], axis=0)
    return out.astype(np.float32)
